# revision 1
# baseline (speedup 1.0000x reference)
"""Mask R-CNN paste_masks_in_image kernel for Trainium2 (8 NeuronCores).

out[n] = Y_n @ mask_n @ X_n  (separable bilinear paste, f32)

 - X_n [28, img_w] column-interp matrix, Y_n [img_h, 28] row-interp matrix
   (<=2 nonzeros per output row/col) are built on host from boxes.
 - Device (per core, 16 instances): mx = maskT.T @ X on TensorE (f32r),
   then rows in 3 permuted 128-row matmuls; only a 384-row full-width
   window per instance is written to HBM via one indirect scatter DMA
   (per-partition row triplets are DRAM-contiguous). Rows outside the
   window are never written: the runner pre-zeros/donates output buffers.
 - Falls back to a dense full-image writer if any box window exceeds the
   static 384-row budget (cannot happen for in-distribution inputs).
"""
import sys

if "/opt/trn_rl_repo" not in sys.path:
    sys.path.insert(0, "/opt/trn_rl_repo")

import numpy as np

N_CORES = 8
HM = WM = 28
PW = 112          # payload partitions
WIN = 3 * PW      # 336-row window; max nonzero span is <=309 rows

_BUILD_CACHE = {}
_ws_ctr = [0]


def _split_multi_waits(nc):
    """This image's walrus allows only ONE sync-wait per instruction; hoist
    extra waits onto preceding NoOps on the same engine."""
    import concourse.mybir as mybir

    for fn in nc.m.functions:
        for blk in fn.blocks:
            insts = list(blk.instructions)
            out = []
            changed = False
            for inst in insts:
                si = getattr(inst, "sync_info", None)
                waits = list(si.on_wait) if (si is not None and si.on_wait) else []
                if len(waits) > 1:
                    changed = True
                    for w in waits[:-1]:
                        _ws_ctr[0] += 1
                        out.append(
                            mybir.InstNoOp(
                                name=f"waitsplit-{_ws_ctr[0]}",
                                engine=inst.engine,
                                sync_info=mybir.SyncInfo(on_wait=[w], on_update=[]),
                            )
                        )
                    si.on_wait = [waits[-1]]
                out.append(inst)
            if changed:
                try:
                    blk.instructions = out
                except Exception:
                    del blk.instructions[:]
                    blk.instructions.extend(out)


def _interp_mats(p0, p1, out_size, mask_size):
    """W[n, k, j] = w0*(i0==k) + w1*(i0+1==k); exact f32 replication of the
    reference's align_corners=False bilinear weights with zero padding."""
    xs = (np.arange(out_size, dtype=np.float32) + np.float32(0.5))[None, :]
    g = (xs - p0[:, None]) / (p1 - p0)[:, None] * np.float32(2) - np.float32(1)
    p = (g + np.float32(1)) * np.float32(mask_size * 0.5) - np.float32(0.5)
    f = np.floor(p)
    i0 = f.astype(np.int64)
    w1 = (p - f).astype(np.float32)
    w0 = np.float32(1.0) - w1
    ks = np.arange(mask_size, dtype=np.int64)[None, :, None]
    W = (i0[:, None, :] == ks) * w0[:, None, :] + ((i0 + 1)[:, None, :] == ks) * w1[
        :, None, :
    ]
    return np.ascontiguousarray(W.astype(np.float32))


def _scaled_boxes(boxes, img_h, img_w, in_h, in_w):
    sx = np.float32(img_w / in_w)
    sy = np.float32(img_h / in_h)
    b = boxes.astype(np.float32) * np.array([sx, sy, sx, sy], np.float32)
    x0 = np.clip(b[:, 0], np.float32(0.0), np.float32(img_w))
    y0 = np.clip(b[:, 1], np.float32(0.0), np.float32(img_h))
    x1 = np.clip(b[:, 2], np.float32(0.0), np.float32(img_w))
    y1 = np.clip(b[:, 3], np.float32(0.0), np.float32(img_h))
    return x0, y0, x1, y1


def _chunks(img_w):
    out = []
    c = 0
    while c < img_w:
        cw = min(512, img_w - c)
        out.append((c, cw))
        c += cw
    return out


def _build_windowed(ni, img_h, img_w):
    import concourse.bass as bass
    import concourse.mybir as mybir
    from concourse.tile import TileContext

    f32 = mybir.dt.float32
    f32r = mybir.dt.float32r
    i32 = mybir.dt.int32
    nc = bass.Bass()
    maskT_d = nc.dram_tensor("maskT", [ni, WM, HM], f32r, kind="ExternalInput")
    x_d = nc.dram_tensor("xmat", [ni, WM, img_w], f32r, kind="ExternalInput")
    ytw_d = nc.dram_tensor("ytwmat", [ni, HM, WIN], f32r, kind="ExternalInput")
    idx_d = nc.dram_tensor("rowidx", [128, ni], i32, kind="ExternalInput")
    out_d = nc.dram_tensor("out", [ni, img_h, img_w], f32, kind="ExternalOutput")
    outv = out_d.rearrange("n h w -> (n h) w")
    chunks = _chunks(img_w)

    with TileContext(nc) as tc:
        with (
            tc.tile_pool(name="w", bufs=4) as wp,
            tc.tile_pool(name="ix", bufs=1) as ixp,
            tc.tile_pool(name="mx", bufs=2) as mxp,
            tc.tile_pool(name="psA", bufs=2, space="PSUM") as psa,
            tc.tile_pool(name="psB", bufs=2, space="PSUM") as psb,
            tc.tile_pool(name="pay", bufs=6) as payp,
        ):
            idxs = ixp.tile([128, ni], i32, tag="idx")
            nc.sync.dma_start(out=idxs[:], in_=idx_d[:])
            for n in range(ni):
                mT = wp.tile([WM, HM], f32r, tag="mT")
                xt = wp.tile([WM, img_w], f32r, tag="xt")
                ytw = wp.tile([HM, WIN], f32r, tag="ytw")
                nc.sync.dma_start(out=mT[:], in_=maskT_d[n])
                nc.sync.dma_start(out=xt[:], in_=x_d[n])
                nc.sync.dma_start(out=ytw[:], in_=ytw_d[n])

                mx = mxp.tile([HM, img_w], f32r, tag="mx")
                for j, (c0, cw) in enumerate(chunks):
                    pa = psa.tile([HM, 512], f32, tag="pa")
                    nc.tensor.matmul(
                        out=pa[:, :cw],
                        lhsT=mT[:],
                        rhs=xt[:, c0 : c0 + cw],
                        start=True,
                        stop=True,
                    )
                    if j % 2 == 0:
                        nc.vector.tensor_copy(out=mx[:, c0 : c0 + cw], in_=pa[:, :cw])
                    else:
                        nc.scalar.copy(out=mx[:, c0 : c0 + cw], in_=pa[:, :cw])

                pay = payp.tile([PW, 3 * img_w], f32, tag="pay")
                for j in range(3):
                    pb = psb.tile([PW, 3 * 512], f32, tag="pb")
                    for k, (c0, cw) in enumerate(chunks):
                        nc.tensor.matmul(
                            out=pb[:, k * 512 : k * 512 + cw],
                            lhsT=ytw[:, j * PW : (j + 1) * PW],
                            rhs=mx[:, c0 : c0 + cw],
                            start=True,
                            stop=True,
                        )
                    for k, (c0, cw) in enumerate(chunks):
                        eng = (
                            nc.vector.tensor_copy if (k + j) % 2 == 0 else nc.scalar.copy
                        )
                        eng(
                            out=pay[:, j * img_w + c0 : j * img_w + c0 + cw],
                            in_=pb[:, k * 512 : k * 512 + cw],
                        )
                nc.gpsimd.indirect_dma_start(
                    out=outv[:],
                    out_offset=bass.IndirectOffsetOnAxis(ap=idxs[:PW, n : n + 1], axis=0),
                    in_=pay[:],
                    in_offset=None,
                )
    _split_multi_waits(nc)
    return nc


def _build_dense(ni, img_h, img_w):
    """Fallback: writes every output pixel (no window assumption)."""
    import concourse.bass as bass
    import concourse.mybir as mybir
    from concourse.tile import TileContext

    f32 = mybir.dt.float32
    f32r = mybir.dt.float32r
    nc = bass.Bass()
    maskT_d = nc.dram_tensor("maskT", [ni, WM, HM], f32r, kind="ExternalInput")
    x_d = nc.dram_tensor("xmat", [ni, WM, img_w], f32r, kind="ExternalInput")
    yt_d = nc.dram_tensor("ytmat", [ni, HM, img_h], f32r, kind="ExternalInput")
    out_d = nc.dram_tensor("out", [ni, img_h, img_w], f32, kind="ExternalOutput")
    chunks = _chunks(img_w)
    rtiles = []
    r = 0
    while r < img_h:
        rh = min(128, img_h - r)
        rtiles.append((r, rh))
        r += rh

    with TileContext(nc) as tc:
        with (
            tc.tile_pool(name="w", bufs=3) as wp,
            tc.tile_pool(name="mx", bufs=2) as mxp,
            tc.tile_pool(name="psA", bufs=2, space="PSUM") as psa,
            tc.tile_pool(name="psB", bufs=2, space="PSUM") as psb,
            tc.tile_pool(name="ob", bufs=4) as obp,
        ):
            for n in range(ni):
                mT = wp.tile([WM, HM], f32r, tag="mT")
                xt = wp.tile([WM, img_w], f32r, tag="xt")
                yt = wp.tile([HM, img_h], f32r, tag="yt")
                nc.sync.dma_start(out=mT[:], in_=maskT_d[n])
                nc.sync.dma_start(out=xt[:], in_=x_d[n])
                nc.sync.dma_start(out=yt[:], in_=yt_d[n])

                mx = mxp.tile([HM, img_w], f32r, tag="mx")
                for j, (c0, cw) in enumerate(chunks):
                    pa = psa.tile([HM, 512], f32, tag="pa")
                    nc.tensor.matmul(
                        out=pa[:, :cw], lhsT=mT[:], rhs=xt[:, c0 : c0 + cw],
                        start=True, stop=True,
                    )
                    if j % 2 == 0:
                        nc.vector.tensor_copy(out=mx[:, c0 : c0 + cw], in_=pa[:, :cw])
                    else:
                        nc.scalar.copy(out=mx[:, c0 : c0 + cw], in_=pa[:, :cw])

                for r0, rh in rtiles:
                    pb = psb.tile([128, 3 * 512], f32, tag="pb")
                    for k, (c0, cw) in enumerate(chunks):
                        nc.tensor.matmul(
                            out=pb[:rh, k * 512 : k * 512 + cw],
                            lhsT=yt[:, r0 : r0 + rh],
                            rhs=mx[:, c0 : c0 + cw],
                            start=True, stop=True,
                        )
                    ob = obp.tile([128, img_w], f32, tag="ob")
                    for k, (c0, cw) in enumerate(chunks):
                        eng = nc.vector.tensor_copy if k % 2 == 0 else nc.scalar.copy
                        eng(out=ob[:rh, c0 : c0 + cw], in_=pb[:rh, k * 512 : k * 512 + cw])
                    nc.sync.dma_start(out=out_d[n, r0 : r0 + rh, :], in_=ob[:rh, :])
    _split_multi_waits(nc)
    return nc


def _prep_common(masks, boxes, img_h, img_w, in_h, in_w):
    x0, y0, x1, y1 = _scaled_boxes(boxes, img_h, img_w, in_h, in_w)
    xmat = _interp_mats(x0, x1, img_w, WM)   # [N, 28, img_w]
    ytmat = _interp_mats(y0, y1, img_h, HM)  # [N, 28, img_h]
    maskt = np.ascontiguousarray(np.transpose(masks[:, 0].astype(np.float32), (0, 2, 1)))
    return maskt, xmat, ytmat


def _windows(ytmat, img_h):
    """Per-instance window start r0 from the actual Yt nonzero columns.
    Returns (r0s, ok): ok False if any instance's span exceeds WIN."""
    n = ytmat.shape[0]
    nz = ytmat.any(axis=1)
    r0s = np.zeros(n, np.int64)
    for i in range(n):
        nzr = np.flatnonzero(nz[i])
        if nzr.size == 0:
            r0s[i] = 0
            continue
        r0 = min(max(int(nzr[0]), 0), max(img_h - WIN, 0))
        if int(nzr[-1]) >= r0 + WIN:
            return r0s, False
        r0s[i] = r0
    return r0s, True


def _run(masks, boxes, img_h, img_w, in_h, in_w, trace=False):
    from concourse.bass_utils import run_bass_kernel_spmd

    n = masks.shape[0]
    assert n % N_CORES == 0
    ni = n // N_CORES
    maskt, xmat, ytmat = _prep_common(masks, boxes, img_h, img_w, in_h, in_w)
    r0s, windowed = _windows(ytmat, img_h)
    windowed = windowed and img_h >= WIN

    if windowed:
        key = ("win", ni, img_h, img_w)
        if key not in _BUILD_CACHE:
            _BUILD_CACHE[key] = _build_windowed(ni, img_h, img_w)
        nc = _BUILD_CACHE[key]
        ytw = np.zeros((n, HM, WIN), np.float32)
        for i in range(n):
            w = ytmat[i][:, r0s[i] : r0s[i] + WIN]
            ytw[i] = np.concatenate([w[:, 0::3], w[:, 1::3], w[:, 2::3]], axis=1)
        in_maps = []
        for c in range(N_CORES):
            s = slice(c * ni, (c + 1) * ni)
            loc = np.arange(ni)
            idx = (
                (loc[None, :] * img_h + r0s[s][None, :]) + 3 * np.arange(128)[:, None]
            ).astype(np.int32)
            idx[PW:] = 0
            in_maps.append(
                {
                    "maskT": maskt[s],
                    "xmat": xmat[s],
                    "ytwmat": ytw[s],
                    "rowidx": np.ascontiguousarray(idx),
                }
            )
    else:
        key = ("dense", ni, img_h, img_w)
        if key not in _BUILD_CACHE:
            _BUILD_CACHE[key] = _build_dense(ni, img_h, img_w)
        nc = _BUILD_CACHE[key]
        in_maps = []
        for c in range(N_CORES):
            s = slice(c * ni, (c + 1) * ni)
            in_maps.append({"maskT": maskt[s], "xmat": xmat[s], "ytmat": ytmat[s]})

    res = run_bass_kernel_spmd(nc, in_maps, core_ids=list(range(N_CORES)), trace=trace)
    out = np.concatenate([res.results[c]["out"] for c in range(N_CORES)], axis=0)
    return out, res


def kernel(masks, boxes, img_h, img_w, in_h, in_w):
    img_h, img_w, in_h, in_w = int(img_h), int(img_w), int(in_h), int(in_w)
    masks = np.asarray(masks, dtype=np.float32)
    boxes = np.asarray(boxes, dtype=np.float32)
    out, _ = _run(masks, boxes, img_h, img_w, in_h, in_w, trace=False)
    return out



# revision 9
# speedup vs baseline: 1.0344x; 1.0344x over previous
"""Mask R-CNN paste_masks_in_image kernel for Trainium2 (8 NeuronCores).

out[n] = Y_n @ mask_n @ X_n  (separable bilinear paste), fp16 device math.

Windowed design (in-distribution boxes span <=308 rows x <=306 cols):
 - Host packs, per group of 4 instances, one fp16 blob [112, 753]:
   block-diag stacked maskT | stacked 320-col X windows | per-instance
   321-row Y windows with columns permuted so window row 3p+b lands on
   (partition p, chunk b).
 - Device: one [112,112]x[112,320] block-diag matmul produces the 4
   stacked mx = mask @ X; per instance, 3 matmuls [28,107]x[28,320]
   produce the 321 window rows as [107, 3*320]; PSUM->SBUF copies cast
   to fp16; one plain contiguous HWDGE DMA per instance stores the
   dense window [107, 960] (1920 B per-partition runs).
 - Host pastes each 321x320 window into the zero f32 canvas at
   (r0, c0). Everything the reference computes (both interp matmuls)
   runs on device; only zero-padding placement happens on host.
 - Falls back to a dense full-image f32 writer if any window budget is
   exceeded (cannot happen for in-distribution inputs).
"""
import sys

if "/opt/trn_rl_repo" not in sys.path:
    sys.path.insert(0, "/opt/trn_rl_repo")

import numpy as np

N_CORES = 8
HM = WM = 28
PWIN = 107        # window payload partitions
RW = 3 * PWIN     # 321-row window (max span 308)
CW = 320          # col window (max span 306)
GI = 4            # instances per matmul group (block-diag mx batch)

_BUILD_CACHE = {}
_ws_ctr = [0]


def _split_multi_waits(nc):
    """This image's walrus allows only ONE sync-wait per instruction; hoist
    extra waits onto preceding NoOps on the same engine."""
    import concourse.mybir as mybir

    for fn in nc.m.functions:
        for blk in fn.blocks:
            insts = list(blk.instructions)
            out = []
            changed = False
            for inst in insts:
                si = getattr(inst, "sync_info", None)
                waits = list(si.on_wait) if (si is not None and si.on_wait) else []
                if len(waits) > 1:
                    changed = True
                    for w in waits[:-1]:
                        _ws_ctr[0] += 1
                        out.append(
                            mybir.InstNoOp(
                                name=f"waitsplit-{_ws_ctr[0]}",
                                engine=inst.engine,
                                sync_info=mybir.SyncInfo(on_wait=[w], on_update=[]),
                            )
                        )
                    si.on_wait = [waits[-1]]
                out.append(inst)
            if changed:
                try:
                    blk.instructions = out
                except Exception:
                    del blk.instructions[:]
                    blk.instructions.extend(out)


def _interp_mats(p0, p1, out_size, mask_size):
    """W[n, k, j] = w0*(i0==k) + w1*(i0+1==k); exact f32 replication of the
    reference's align_corners=False bilinear weights with zero padding."""
    xs = (np.arange(out_size, dtype=np.float32) + np.float32(0.5))[None, :]
    g = (xs - p0[:, None]) / (p1 - p0)[:, None] * np.float32(2) - np.float32(1)
    p = (g + np.float32(1)) * np.float32(mask_size * 0.5) - np.float32(0.5)
    f = np.floor(p)
    i0 = f.astype(np.int64)
    w1 = (p - f).astype(np.float32)
    w0 = np.float32(1.0) - w1
    ks = np.arange(mask_size, dtype=np.int64)[None, :, None]
    W = (i0[:, None, :] == ks) * w0[:, None, :] + ((i0 + 1)[:, None, :] == ks) * w1[
        :, None, :
    ]
    return np.ascontiguousarray(W.astype(np.float32))


def _scaled_boxes(boxes, img_h, img_w, in_h, in_w):
    sx = np.float32(img_w / in_w)
    sy = np.float32(img_h / in_h)
    b = boxes.astype(np.float32) * np.array([sx, sy, sx, sy], np.float32)
    x0 = np.clip(b[:, 0], np.float32(0.0), np.float32(img_w))
    y0 = np.clip(b[:, 1], np.float32(0.0), np.float32(img_h))
    x1 = np.clip(b[:, 2], np.float32(0.0), np.float32(img_w))
    y1 = np.clip(b[:, 3], np.float32(0.0), np.float32(img_h))
    return x0, y0, x1, y1


def _chunks(img_w):
    out = []
    c = 0
    while c < img_w:
        cw = min(512, img_w - c)
        out.append((c, cw))
        c += cw
    return out


def _build_win(ni):
    """ni instances (multiple of GI) per core, fp16 windowed paste."""
    import concourse.bass as bass
    import concourse.mybir as mybir
    from concourse.tile import TileContext

    f16 = mybir.dt.float16
    f32 = mybir.dt.float32
    ngrp = ni // GI
    iw = 112 + CW + 3 * GI * PWIN  # 1716 input cols per group
    nc = bass.Bass()
    inp_d = nc.dram_tensor("inp", [ngrp, 112, iw], f16, kind="ExternalInput")
    outw_d = nc.dram_tensor("outw", [ni, PWIN, 3 * CW], f16, kind="ExternalOutput")

    with TileContext(nc) as tc:
        with (
            tc.tile_pool(name="w", bufs=2) as wp,
            tc.tile_pool(name="mx", bufs=2) as mxp,
            tc.tile_pool(name="psA", bufs=2, space="PSUM") as psa,
            tc.tile_pool(name="psB", bufs=4, space="PSUM") as psb,
            tc.tile_pool(name="pay", bufs=4) as payp,
        ):
            for g in range(ngrp):
                inp = wp.tile([112, iw], f16, tag="inp")
                nc.sync.dma_start(out=inp[:], in_=inp_d[g])

                pa = psa.tile([112, CW], f32, tag="pa")
                nc.tensor.matmul(
                    out=pa[:],
                    lhsT=inp[:, 0:112],
                    rhs=inp[:, 112 : 112 + CW],
                    start=True,
                    stop=True,
                )
                mx = mxp.tile([112, CW], f16, tag="mx")
                nc.vector.tensor_copy(out=mx[:], in_=pa[:])

                for nloc in range(GI):
                    n = g * GI + nloc
                    pay = payp.tile([PWIN, 3 * CW], f16, tag="pay")
                    for b in range(3):
                        pb = psb.tile([PWIN, CW], f32, tag="pb")
                        c0 = 112 + CW + (3 * nloc + b) * PWIN
                        nc.tensor.matmul(
                            out=pb[:],
                            lhsT=inp[:, c0 : c0 + PWIN],
                            rhs=mx[:],
                            start=True,
                            stop=True,
                        )
                        eng = (
                            nc.vector.tensor_copy
                            if (nloc + b) % 2 == 0
                            else nc.scalar.copy
                        )
                        eng(out=pay[:, b * CW : (b + 1) * CW], in_=pb[:])
                    nc.sync.dma_start(out=outw_d[n], in_=pay[:])
    _split_multi_waits(nc)
    return nc


def _build_dense(ni, img_h, img_w):
    """Fallback: writes every output pixel (no window assumption), f32."""
    import concourse.bass as bass
    import concourse.mybir as mybir
    from concourse.tile import TileContext

    f32 = mybir.dt.float32
    f32r = mybir.dt.float32r
    nc = bass.Bass()
    maskT_d = nc.dram_tensor("maskT", [ni, WM, HM], f32r, kind="ExternalInput")
    x_d = nc.dram_tensor("xmat", [ni, WM, img_w], f32r, kind="ExternalInput")
    yt_d = nc.dram_tensor("ytmat", [ni, HM, img_h], f32r, kind="ExternalInput")
    out_d = nc.dram_tensor("out", [ni, img_h, img_w], f32, kind="ExternalOutput")
    chunks = _chunks(img_w)
    rtiles = []
    r = 0
    while r < img_h:
        rh = min(128, img_h - r)
        rtiles.append((r, rh))
        r += rh

    with TileContext(nc) as tc:
        with (
            tc.tile_pool(name="w", bufs=3) as wp,
            tc.tile_pool(name="mx", bufs=2) as mxp,
            tc.tile_pool(name="psA", bufs=2, space="PSUM") as psa,
            tc.tile_pool(name="psB", bufs=2, space="PSUM") as psb,
            tc.tile_pool(name="ob", bufs=4) as obp,
        ):
            for n in range(ni):
                mT = wp.tile([WM, HM], f32r, tag="mT")
                xt = wp.tile([WM, img_w], f32r, tag="xt")
                yt = wp.tile([HM, img_h], f32r, tag="yt")
                nc.sync.dma_start(out=mT[:], in_=maskT_d[n])
                nc.sync.dma_start(out=xt[:], in_=x_d[n])
                nc.sync.dma_start(out=yt[:], in_=yt_d[n])

                mx = mxp.tile([HM, img_w], f32r, tag="mx")
                for j, (c0, cw) in enumerate(chunks):
                    pa = psa.tile([HM, 512], f32, tag="pa")
                    nc.tensor.matmul(
                        out=pa[:, :cw], lhsT=mT[:], rhs=xt[:, c0 : c0 + cw],
                        start=True, stop=True,
                    )
                    if j % 2 == 0:
                        nc.vector.tensor_copy(out=mx[:, c0 : c0 + cw], in_=pa[:, :cw])
                    else:
                        nc.scalar.copy(out=mx[:, c0 : c0 + cw], in_=pa[:, :cw])

                for r0, rh in rtiles:
                    pb = psb.tile([128, 3 * 512], f32, tag="pb")
                    for k, (c0, cw) in enumerate(chunks):
                        nc.tensor.matmul(
                            out=pb[:rh, k * 512 : k * 512 + cw],
                            lhsT=yt[:, r0 : r0 + rh],
                            rhs=mx[:, c0 : c0 + cw],
                            start=True, stop=True,
                        )
                    ob = obp.tile([128, img_w], f32, tag="ob")
                    for k, (c0, cw) in enumerate(chunks):
                        eng = nc.vector.tensor_copy if k % 2 == 0 else nc.scalar.copy
                        eng(out=ob[:rh, c0 : c0 + cw], in_=pb[:rh, k * 512 : k * 512 + cw])
                    nc.sync.dma_start(out=out_d[n, r0 : r0 + rh, :], in_=ob[:rh, :])
    _split_multi_waits(nc)
    return nc


def _prep_common(masks, boxes, img_h, img_w, in_h, in_w):
    x0, y0, x1, y1 = _scaled_boxes(boxes, img_h, img_w, in_h, in_w)
    xmat = _interp_mats(x0, x1, img_w, WM)   # [N, 28, img_w]
    ytmat = _interp_mats(y0, y1, img_h, HM)  # [N, 28, img_h]
    return xmat, ytmat


def _windows(mat, img_size, win):
    """Per-instance window start from actual nonzero columns of mat
    [N, 28, img_size]. Returns (starts, ok)."""
    n = mat.shape[0]
    nz = mat.any(axis=1)
    starts = np.zeros(n, np.int64)
    if img_size < win:
        return starts, False
    for i in range(n):
        nzc = np.flatnonzero(nz[i])
        if nzc.size == 0:
            continue
        s = min(max(int(nzc[0]), 0), img_size - win)
        if int(nzc[-1]) >= s + win:
            return starts, False
        starts[i] = s
    return starts, True


def _prep_win(masks, xmat, ytmat, r0s, c0s, ni):
    """Build per-core fp16 input blobs."""
    n = masks.shape[0]
    ncores = n // ni
    ngrp = ni // GI
    iw = 112 + CW + 3 * GI * PWIN
    inp = np.zeros((ncores, ngrp, 112, iw), np.float16)
    for c in range(ncores):
        for g in range(ngrp):
            blob = inp[c, g]
            for b in range(GI):
                nn = c * ni + g * GI + b
                sl = slice(28 * b, 28 * b + 28)
                blob[sl, sl] = masks[nn, 0].T.astype(np.float16)
                blob[sl, 112 : 112 + CW] = xmat[nn][
                    :, c0s[nn] : c0s[nn] + CW
                ].astype(np.float16)
                # matmul chunk (b, j) covers window rows j*PWIN..j*PWIN+106
                # on partitions 0..106; host reassembles r = j*PWIN + p
                ytp = ytmat[nn][:, r0s[nn] : r0s[nn] + RW].astype(np.float16)
                blob[sl, 112 + CW + 3 * b * PWIN : 112 + CW + 3 * (b + 1) * PWIN] = ytp
    return inp


def _run(masks, boxes, img_h, img_w, in_h, in_w, trace=False):
    from concourse.bass_utils import run_bass_kernel_spmd

    n = masks.shape[0]
    assert n % N_CORES == 0
    ni = n // N_CORES
    xmat, ytmat = _prep_common(masks, boxes, img_h, img_w, in_h, in_w)
    c0s, okc = _windows(xmat, img_w, CW)
    r0s, okr = _windows(ytmat, img_h, RW)
    windowed = okc and okr and ni % GI == 0

    if windowed:
        key = ("win", ni)
        if key not in _BUILD_CACHE:
            _BUILD_CACHE[key] = _build_win(ni)
        nc = _BUILD_CACHE[key]
        inp = _prep_win(masks, xmat, ytmat, r0s, c0s, ni)
        in_maps = [{"inp": np.ascontiguousarray(inp[c])} for c in range(N_CORES)]
        res = run_bass_kernel_spmd(
            nc, in_maps, core_ids=list(range(N_CORES)), trace=trace
        )
        out = np.zeros((n, img_h, img_w), np.float32)
        for c in range(N_CORES):
            wins = (
                np.asarray(res.results[c]["outw"])
                .reshape(ni, PWIN, 3, CW)
                .transpose(0, 2, 1, 3)
                .reshape(ni, RW, CW)
                .astype(np.float32)
            )
            for i in range(ni):
                nn = c * ni + i
                out[nn, r0s[nn] : r0s[nn] + RW, c0s[nn] : c0s[nn] + CW] = wins[i]
        return out, res

    key = ("dense", ni, img_h, img_w)
    if key not in _BUILD_CACHE:
        _BUILD_CACHE[key] = _build_dense(ni, img_h, img_w)
    nc = _BUILD_CACHE[key]
    maskt = np.ascontiguousarray(
        np.transpose(masks[:, 0].astype(np.float32), (0, 2, 1))
    )
    in_maps = []
    for c in range(N_CORES):
        s = slice(c * ni, (c + 1) * ni)
        in_maps.append({"maskT": maskt[s], "xmat": xmat[s], "ytmat": ytmat[s]})
    res = run_bass_kernel_spmd(nc, in_maps, core_ids=list(range(N_CORES)), trace=trace)
    out = np.concatenate(
        [np.asarray(res.results[c]["out"]) for c in range(N_CORES)], axis=0
    ).astype(np.float32)
    return out, res


def kernel(masks, boxes, img_h, img_w, in_h, in_w):
    img_h, img_w, in_h, in_w = int(img_h), int(img_w), int(in_h), int(in_w)
    masks = np.asarray(masks, dtype=np.float32)
    boxes = np.asarray(boxes, dtype=np.float32)
    out, _ = _run(masks, boxes, img_h, img_w, in_h, in_w, trace=False)
    return out


# revision 11
# speedup vs baseline: 1.3982x; 1.3517x over previous
"""Mask R-CNN paste_masks_in_image kernel for Trainium2 (8 NeuronCores).

out[n] = Y_n @ mask_n @ X_n  (separable bilinear paste), fp16 device math.

Windowed design (in-distribution boxes span <=308 rows x <=306 cols):
 - Host packs, per group of 4 instances, one fp16 blob [112, 753]:
   block-diag stacked maskT | stacked 320-col X windows | per-instance
   321-row Y windows with columns permuted so window row 3p+b lands on
   (partition p, chunk b).
 - Device: one [112,112]x[112,320] block-diag matmul produces the 4
   stacked mx = mask @ X; per instance, 3 matmuls [28,107]x[28,320]
   produce the 321 window rows as [107, 3*320]; PSUM->SBUF copies cast
   to fp16; one plain contiguous HWDGE DMA per instance stores the
   dense window [107, 960] (1920 B per-partition runs).
 - Host pastes each 321x320 window into the zero f32 canvas at
   (r0, c0). Everything the reference computes (both interp matmuls)
   runs on device; only zero-padding placement happens on host.
 - Falls back to a dense full-image f32 writer if any window budget is
   exceeded (cannot happen for in-distribution inputs).
"""
import sys

if "/opt/trn_rl_repo" not in sys.path:
    sys.path.insert(0, "/opt/trn_rl_repo")

import numpy as np

N_CORES = 8
HM = WM = 28
PWIN = 107        # window payload partitions
RW = 3 * PWIN     # 321-row window (max span 308)
CW = 320          # col window (max span 306)
GI = 4            # instances per matmul group (block-diag mx batch)

_BUILD_CACHE = {}
_ws_ctr = [0]


def _split_multi_waits(nc):
    """This image's walrus allows only ONE sync-wait per instruction; hoist
    extra waits onto preceding NoOps on the same engine."""
    import concourse.mybir as mybir

    for fn in nc.m.functions:
        for blk in fn.blocks:
            insts = list(blk.instructions)
            out = []
            changed = False
            for inst in insts:
                si = getattr(inst, "sync_info", None)
                waits = list(si.on_wait) if (si is not None and si.on_wait) else []
                if len(waits) > 1:
                    changed = True
                    for w in waits[:-1]:
                        _ws_ctr[0] += 1
                        out.append(
                            mybir.InstNoOp(
                                name=f"waitsplit-{_ws_ctr[0]}",
                                engine=inst.engine,
                                sync_info=mybir.SyncInfo(on_wait=[w], on_update=[]),
                            )
                        )
                    si.on_wait = [waits[-1]]
                out.append(inst)
            if changed:
                try:
                    blk.instructions = out
                except Exception:
                    del blk.instructions[:]
                    blk.instructions.extend(out)


def _interp_mats(p0, p1, out_size, mask_size):
    """W[n, k, j] = w0*(i0==k) + w1*(i0+1==k); exact f32 replication of the
    reference's align_corners=False bilinear weights with zero padding."""
    xs = (np.arange(out_size, dtype=np.float32) + np.float32(0.5))[None, :]
    g = (xs - p0[:, None]) / (p1 - p0)[:, None] * np.float32(2) - np.float32(1)
    p = (g + np.float32(1)) * np.float32(mask_size * 0.5) - np.float32(0.5)
    f = np.floor(p)
    i0 = f.astype(np.int64)
    w1 = (p - f).astype(np.float32)
    w0 = np.float32(1.0) - w1
    ks = np.arange(mask_size, dtype=np.int64)[None, :, None]
    W = (i0[:, None, :] == ks) * w0[:, None, :] + ((i0 + 1)[:, None, :] == ks) * w1[
        :, None, :
    ]
    return np.ascontiguousarray(W.astype(np.float32))


def _scaled_boxes(boxes, img_h, img_w, in_h, in_w):
    sx = np.float32(img_w / in_w)
    sy = np.float32(img_h / in_h)
    b = boxes.astype(np.float32) * np.array([sx, sy, sx, sy], np.float32)
    x0 = np.clip(b[:, 0], np.float32(0.0), np.float32(img_w))
    y0 = np.clip(b[:, 1], np.float32(0.0), np.float32(img_h))
    x1 = np.clip(b[:, 2], np.float32(0.0), np.float32(img_w))
    y1 = np.clip(b[:, 3], np.float32(0.0), np.float32(img_h))
    return x0, y0, x1, y1


def _chunks(img_w):
    out = []
    c = 0
    while c < img_w:
        cw = min(512, img_w - c)
        out.append((c, cw))
        c += cw
    return out


def _build_win(ni):
    """ni instances (multiple of GI) per core, fp16 windowed paste."""
    import concourse.bass as bass
    import concourse.mybir as mybir
    from concourse.tile import TileContext

    f16 = mybir.dt.float16
    f32 = mybir.dt.float32
    ngrp = ni // GI
    iw = 112 + CW + 3 * GI * PWIN  # 1716 input cols per group
    nc = bass.Bass()
    inp_d = nc.dram_tensor("inp", [ngrp, 112, iw], f16, kind="ExternalInput")
    outw_d = nc.dram_tensor("outw", [ni, PWIN, 3 * CW], f16, kind="ExternalOutput")
    # [PWIN, ni, 3*CW]: per-group stores keep per-partition runs contiguous
    outw_v = outw_d.rearrange("n p w -> p n w")

    with TileContext(nc) as tc:
        with (
            tc.tile_pool(name="w", bufs=2) as wp,
            tc.tile_pool(name="mx", bufs=2) as mxp,
            tc.tile_pool(name="psA", bufs=2, space="PSUM") as psa,
            tc.tile_pool(name="psB", bufs=4, space="PSUM") as psb,
            tc.tile_pool(name="pay", bufs=4) as payp,
        ):
            for g in range(ngrp):
                inp = wp.tile([112, iw], f16, tag="inp")
                nc.sync.dma_start(out=inp[:], in_=inp_d[g])

                pa = psa.tile([112, CW], f32, tag="pa")
                nc.tensor.matmul(
                    out=pa[:],
                    lhsT=inp[:, 0:112],
                    rhs=inp[:, 112 : 112 + CW],
                    start=True,
                    stop=True,
                )
                mx = mxp.tile([112, CW], f16, tag="mx")
                nc.vector.tensor_copy(out=mx[:], in_=pa[:])

                pay = payp.tile([PWIN, GI * 3 * CW], f16, tag="pay")
                for nloc in range(GI):
                    for b in range(3):
                        pb = psb.tile([PWIN, CW], f32, tag="pb")
                        k = 3 * nloc + b
                        c0 = 112 + CW + k * PWIN
                        nc.tensor.matmul(
                            out=pb[:],
                            lhsT=inp[:, c0 : c0 + PWIN],
                            rhs=mx[:],
                            start=True,
                            stop=True,
                        )
                        eng = nc.vector.tensor_copy if k % 2 == 0 else nc.scalar.copy
                        eng(out=pay[:, k * CW : (k + 1) * CW], in_=pb[:])
                nc.gpsimd.dma_start(
                    out=outw_v[:, g * GI : (g + 1) * GI, :], in_=pay[:]
                )
    _split_multi_waits(nc)
    return nc


def _build_dense(ni, img_h, img_w):
    """Fallback: writes every output pixel (no window assumption), f32."""
    import concourse.bass as bass
    import concourse.mybir as mybir
    from concourse.tile import TileContext

    f32 = mybir.dt.float32
    f32r = mybir.dt.float32r
    nc = bass.Bass()
    maskT_d = nc.dram_tensor("maskT", [ni, WM, HM], f32r, kind="ExternalInput")
    x_d = nc.dram_tensor("xmat", [ni, WM, img_w], f32r, kind="ExternalInput")
    yt_d = nc.dram_tensor("ytmat", [ni, HM, img_h], f32r, kind="ExternalInput")
    out_d = nc.dram_tensor("out", [ni, img_h, img_w], f32, kind="ExternalOutput")
    chunks = _chunks(img_w)
    rtiles = []
    r = 0
    while r < img_h:
        rh = min(128, img_h - r)
        rtiles.append((r, rh))
        r += rh

    with TileContext(nc) as tc:
        with (
            tc.tile_pool(name="w", bufs=3) as wp,
            tc.tile_pool(name="mx", bufs=2) as mxp,
            tc.tile_pool(name="psA", bufs=2, space="PSUM") as psa,
            tc.tile_pool(name="psB", bufs=2, space="PSUM") as psb,
            tc.tile_pool(name="ob", bufs=4) as obp,
        ):
            for n in range(ni):
                mT = wp.tile([WM, HM], f32r, tag="mT")
                xt = wp.tile([WM, img_w], f32r, tag="xt")
                yt = wp.tile([HM, img_h], f32r, tag="yt")
                nc.sync.dma_start(out=mT[:], in_=maskT_d[n])
                nc.sync.dma_start(out=xt[:], in_=x_d[n])
                nc.sync.dma_start(out=yt[:], in_=yt_d[n])

                mx = mxp.tile([HM, img_w], f32r, tag="mx")
                for j, (c0, cw) in enumerate(chunks):
                    pa = psa.tile([HM, 512], f32, tag="pa")
                    nc.tensor.matmul(
                        out=pa[:, :cw], lhsT=mT[:], rhs=xt[:, c0 : c0 + cw],
                        start=True, stop=True,
                    )
                    if j % 2 == 0:
                        nc.vector.tensor_copy(out=mx[:, c0 : c0 + cw], in_=pa[:, :cw])
                    else:
                        nc.scalar.copy(out=mx[:, c0 : c0 + cw], in_=pa[:, :cw])

                for r0, rh in rtiles:
                    pb = psb.tile([128, 3 * 512], f32, tag="pb")
                    for k, (c0, cw) in enumerate(chunks):
                        nc.tensor.matmul(
                            out=pb[:rh, k * 512 : k * 512 + cw],
                            lhsT=yt[:, r0 : r0 + rh],
                            rhs=mx[:, c0 : c0 + cw],
                            start=True, stop=True,
                        )
                    ob = obp.tile([128, img_w], f32, tag="ob")
                    for k, (c0, cw) in enumerate(chunks):
                        eng = nc.vector.tensor_copy if k % 2 == 0 else nc.scalar.copy
                        eng(out=ob[:rh, c0 : c0 + cw], in_=pb[:rh, k * 512 : k * 512 + cw])
                    nc.sync.dma_start(out=out_d[n, r0 : r0 + rh, :], in_=ob[:rh, :])
    _split_multi_waits(nc)
    return nc


def _prep_common(masks, boxes, img_h, img_w, in_h, in_w):
    x0, y0, x1, y1 = _scaled_boxes(boxes, img_h, img_w, in_h, in_w)
    xmat = _interp_mats(x0, x1, img_w, WM)   # [N, 28, img_w]
    ytmat = _interp_mats(y0, y1, img_h, HM)  # [N, 28, img_h]
    return xmat, ytmat


def _windows(mat, img_size, win):
    """Per-instance window start from actual nonzero columns of mat
    [N, 28, img_size]. Returns (starts, ok)."""
    n = mat.shape[0]
    nz = mat.any(axis=1)
    starts = np.zeros(n, np.int64)
    if img_size < win:
        return starts, False
    for i in range(n):
        nzc = np.flatnonzero(nz[i])
        if nzc.size == 0:
            continue
        s = min(max(int(nzc[0]), 0), img_size - win)
        if int(nzc[-1]) >= s + win:
            return starts, False
        starts[i] = s
    return starts, True


def _prep_win(masks, xmat, ytmat, r0s, c0s, ni):
    """Build per-core fp16 input blobs."""
    n = masks.shape[0]
    ncores = n // ni
    ngrp = ni // GI
    iw = 112 + CW + 3 * GI * PWIN
    inp = np.zeros((ncores, ngrp, 112, iw), np.float16)
    for c in range(ncores):
        for g in range(ngrp):
            blob = inp[c, g]
            for b in range(GI):
                nn = c * ni + g * GI + b
                sl = slice(28 * b, 28 * b + 28)
                blob[sl, sl] = masks[nn, 0].T.astype(np.float16)
                blob[sl, 112 : 112 + CW] = xmat[nn][
                    :, c0s[nn] : c0s[nn] + CW
                ].astype(np.float16)
                # matmul chunk (b, j) covers window rows j*PWIN..j*PWIN+106
                # on partitions 0..106; host reassembles r = j*PWIN + p
                ytp = ytmat[nn][:, r0s[nn] : r0s[nn] + RW].astype(np.float16)
                blob[sl, 112 + CW + 3 * b * PWIN : 112 + CW + 3 * (b + 1) * PWIN] = ytp
    return inp


def _run(masks, boxes, img_h, img_w, in_h, in_w, trace=False):
    from concourse.bass_utils import run_bass_kernel_spmd

    n = masks.shape[0]
    assert n % N_CORES == 0
    ni = n // N_CORES
    xmat, ytmat = _prep_common(masks, boxes, img_h, img_w, in_h, in_w)
    c0s, okc = _windows(xmat, img_w, CW)
    r0s, okr = _windows(ytmat, img_h, RW)
    windowed = okc and okr and ni % GI == 0

    if windowed:
        key = ("win", ni)
        if key not in _BUILD_CACHE:
            _BUILD_CACHE[key] = _build_win(ni)
        nc = _BUILD_CACHE[key]
        inp = _prep_win(masks, xmat, ytmat, r0s, c0s, ni)
        in_maps = [{"inp": np.ascontiguousarray(inp[c])} for c in range(N_CORES)]
        res = run_bass_kernel_spmd(
            nc, in_maps, core_ids=list(range(N_CORES)), trace=trace
        )
        out = np.zeros((n, img_h, img_w), np.float32)
        for c in range(N_CORES):
            wins = (
                np.asarray(res.results[c]["outw"])
                .reshape(ni, PWIN, 3, CW)
                .transpose(0, 2, 1, 3)
                .reshape(ni, RW, CW)
                .astype(np.float32)
            )
            for i in range(ni):
                nn = c * ni + i
                out[nn, r0s[nn] : r0s[nn] + RW, c0s[nn] : c0s[nn] + CW] = wins[i]
        return out, res

    key = ("dense", ni, img_h, img_w)
    if key not in _BUILD_CACHE:
        _BUILD_CACHE[key] = _build_dense(ni, img_h, img_w)
    nc = _BUILD_CACHE[key]
    maskt = np.ascontiguousarray(
        np.transpose(masks[:, 0].astype(np.float32), (0, 2, 1))
    )
    in_maps = []
    for c in range(N_CORES):
        s = slice(c * ni, (c + 1) * ni)
        in_maps.append({"maskT": maskt[s], "xmat": xmat[s], "ytmat": ytmat[s]})
    res = run_bass_kernel_spmd(nc, in_maps, core_ids=list(range(N_CORES)), trace=trace)
    out = np.concatenate(
        [np.asarray(res.results[c]["out"]) for c in range(N_CORES)], axis=0
    ).astype(np.float32)
    return out, res


def kernel(masks, boxes, img_h, img_w, in_h, in_w):
    img_h, img_w, in_h, in_w = int(img_h), int(img_w), int(in_h), int(in_w)
    masks = np.asarray(masks, dtype=np.float32)
    boxes = np.asarray(boxes, dtype=np.float32)
    out, _ = _run(masks, boxes, img_h, img_w, in_h, in_w, trace=False)
    return out
